# revision 7
# baseline (speedup 1.0000x reference)
"""Bahdanau attention w/ coverage — Trainium2 Bass kernel, 8 NeuronCores.

Data-parallel over batch: each core handles 8 rows of the batch.
Per row (T=2048, D=512), processed in 16 chunks of 128 timesteps:
  f[t,e]   = sum_d h[t,d]*Wh[e,d] + dec_b[e] + cov[t]*Wc[e]   (PSUM accum:
             one K=2 matmul for the rank-2 dec/cov part + 4 K=128 bf16
             matmuls with hT tiles obtained via DMA-xbar transpose)
  e_t[t]   = sum_e tanh(f[t,e]) * V[e]      (ACT tanh + DVE fused mul-reduce)
  a        = exp(e_t) / Z                    (no max-subtraction: logits are
             tiny by construction; Z via ones-matmul partition reduce)
  context  = (w @ h) / Z                     (fp32 PSUM accum over resident h)
h is read from HBM exactly once per core.
"""

import numpy as np
from contextlib import ExitStack

import concourse.bass as bass
import concourse.tile as tile
from concourse import mybir
from concourse.masks import make_identity
from concourse.bass_utils import run_bass_kernel_spmd

B, T, D = 64, 2048, 512
NCORES = 8
ROWS = B // NCORES   # 8 batch rows per core
P = 128
CH = T // P          # 16 chunks per row
DB = D // P          # 4 d-blocks
KB = (2 * D) // P    # 8 k-blocks of s_t
F32 = mybir.dt.float32
BF16 = mybir.dt.bfloat16
AF = mybir.ActivationFunctionType
ALU = mybir.AluOpType


def _bcast(ap, n):
    """Partition-broadcast a 1-D (or [1, N]) DRAM AP to n partitions."""
    while len(ap.shape) > 1 and ap.shape[0] == 1:
        ap = ap[0]
    return bass.AP(tensor=ap.tensor, offset=ap.offset, ap=[[0, n]] + list(ap.ap))


def fix_multiwait(nc):
    """This walrus build allows a single sem-wait per instruction; Tile's
    tail drain packs the whole residual vector clock onto one Drain.
    Hoist excess waits onto injected same-engine NoOps."""
    for f in nc.m.functions:
        for bb in f.blocks:
            insts = bb.instructions
            new = []
            changed = False
            for inst in insts:
                si = inst.sync_info
                if si is not None and si.on_wait and len(si.on_wait) > 1:
                    waits = list(si.on_wait)
                    for k, w in enumerate(waits[:-1]):
                        nop = mybir.InstNoOp(
                            name=f"{inst.name}-hoistw{k}", ins=[], outs=[]
                        )
                        nop.engine = inst.engine
                        nop.sync_info = mybir.SyncInfo(on_wait=[w], on_update=[])
                        new.append(nop)
                    si.on_wait = waits[-1:]
                    changed = True
                new.append(inst)
            if changed:
                bb.instructions = new


def build_body(nc, tc, ctx, rows, h, s_t, cov, Wh, Ws, Wsb, Wc, V,
               octx, oa, oncov, reps=1):
    consts = ctx.enter_context(tc.tile_pool(name="consts", bufs=1))
    stage = ctx.enter_context(tc.tile_pool(name="stage", bufs=2))
    psumA = ctx.enter_context(tc.tile_pool(name="psumA", bufs=3, space="PSUM"))
    psumC = ctx.enter_context(tc.tile_pool(name="psumC", bufs=1, space="PSUM"))
    psumS = ctx.enter_context(tc.tile_pool(name="psumS", bufs=1, space="PSUM"))

    # ---------------- one-time setup ----------------
    ident = consts.tile([P, P], F32, tag="ident")
    make_identity(nc, ident)
    ones_col = consts.tile([P, 1], F32, tag="ones_col")
    nc.vector.memset(ones_col, 1.0)
    ones_row = consts.tile([1, P], F32, tag="ones_row")
    nc.vector.memset(ones_row, 1.0)

    # V broadcast to all partitions, bf16
    v_f32 = consts.tile([P, D], F32, tag="v_f32")
    nc.sync.dma_start(out=v_f32, in_=_bcast(V, P))
    v_bf = consts.tile([P, D], BF16, tag="v_bf")
    nc.gpsimd.tensor_copy(out=v_bf, in_=v_f32)

    # WhT (bf16): WhT[j][d, e] with d in block j ; Wh[e, d] in HBM
    wht = [consts.tile([P, D], BF16, tag=f"wht{j}", name=f"wht{j}") for j in range(DB)]
    for i in range(DB):
        wh_stage = stage.tile([P, D], F32, tag="wh_stage")
        nc.sync.dma_start(out=wh_stage, in_=Wh[i * P:(i + 1) * P, :])
        wh_bf = stage.tile([P, D], BF16, tag="wh_bf")
        nc.gpsimd.tensor_copy(out=wh_bf, in_=wh_stage)
        for j in range(DB):
            nc.scalar.dma_start_transpose(
                out=wht[j][:, i * P:(i + 1) * P],
                in_=wh_bf[:, j * P:(j + 1) * P],
            )

    # WsT (bf16): WsT[k][kk, e]; Ws[e, k] in HBM  [512, 1024]
    wst = [consts.tile([P, D], BF16, tag=f"wst{k}", name=f"wst{k}") for k in range(KB)]
    for i in range(DB):
        ws_stage = stage.tile([P, 2 * D], F32, tag="ws_stage")
        nc.sync.dma_start(out=ws_stage, in_=Ws[i * P:(i + 1) * P, :])
        ws_bf = stage.tile([P, 2 * D], BF16, tag="ws_bf")
        nc.gpsimd.tensor_copy(out=ws_bf, in_=ws_stage)
        for k in range(KB):
            nc.scalar.dma_start_transpose(
                out=wst[k][:, i * P:(i + 1) * P],
                in_=ws_bf[:, k * P:(k + 1) * P],
            )

    # s_t transposed: [128k, rows] per k-block (tiny strided DMA), bf16
    stt_bf = []
    for k in range(KB):
        stt_f = stage.tile([P, rows], F32, tag="stt_f")
        nc.sync.dma_start(
            out=stt_f, in_=s_t[:, k * P:(k + 1) * P].rearrange("a b -> b a")
        )
        t_bf = consts.tile([P, rows], BF16, tag=f"stt{k}", name=f"stt{k}")
        nc.gpsimd.tensor_copy(out=t_bf, in_=stt_f)
        stt_bf.append(t_bf)

    # dec[b, e] = s_t[b] @ Ws.T + Ws_b   -> [rows, 512] fp32
    psum_dec = psumS.tile([rows, D], F32, tag="dec")
    for k in range(KB):
        nc.tensor.matmul(psum_dec, lhsT=stt_bf[k], rhs=wst[k],
                         start=(k == 0), stop=(k == KB - 1))
    wsb_bc = consts.tile([rows, D], F32, tag="wsb_bc")
    nc.sync.dma_start(out=wsb_bc, in_=_bcast(Wsb, rows))
    dec_f = consts.tile([rows, D], F32, tag="dec_f")
    nc.vector.tensor_add(dec_f, psum_dec, wsb_bc)
    dec_bf = consts.tile([rows, D], BF16, tag="dec_bf")
    nc.gpsimd.tensor_copy(out=dec_bf, in_=dec_f)

    # Wc row (bf16)
    wc_f = consts.tile([1, D], F32, tag="wc_f")
    nc.sync.dma_start(out=wc_f, in_=Wc.rearrange("a b -> b a"))
    wc_bf = consts.tile([1, D], BF16, tag="wc_bf")
    nc.gpsimd.tensor_copy(out=wc_bf, in_=wc_f)

    # rhs2_all[2, rows*D]: per-row moving operand of the K=2 rank-2 matmul
    # partition 0 = dec_b, partition 1 = WcT
    rhs2 = consts.tile([2, rows * D], BF16, tag="rhs2")
    for b in range(rows):
        nc.sync.dma_start(out=rhs2[0:1, b * D:(b + 1) * D], in_=dec_bf[b:b + 1, :])
        nc.sync.dma_start(out=rhs2[1:2, b * D:(b + 1) * D], in_=wc_bf)

    # main-loop pools (created after setup emission; QC = chunks per group)
    QC = 4
    hrowpool = ctx.enter_context(tc.tile_pool(name="hrow", bufs=2))
    hbfpool = ctx.enter_context(tc.tile_pool(name="hbf", bufs=3))
    htpool = ctx.enter_context(tc.tile_pool(name="ht", bufs=3))
    tfpool = ctx.enter_context(tc.tile_pool(name="tf", bufs=3))
    scpool = ctx.enter_context(tc.tile_pool(name="sc", bufs=1))
    rowpool = ctx.enter_context(tc.tile_pool(name="rowpool", bufs=2))

    def emit_rows():
        for r in range(rows):
            # cov2: stationary [2, T] for the K=2 matmul (row0 ones, row1 cov)
            cov2 = rowpool.tile([2, T], BF16, tag="cov2")
            nc.vector.memset(cov2[0:1, :], 1.0)
            cov_f = rowpool.tile([1, T], F32, tag="cov_f", bufs=1)
            nc.sync.dma_start(out=cov_f, in_=cov[r:r + 1, :])
            cov_bf = rowpool.tile([1, T], BF16, tag="cov_bf", bufs=1)
            nc.gpsimd.tensor_copy(out=cov_bf, in_=cov_f)
            nc.sync.dma_start(out=cov2[1:2, :], in_=cov_bf)
            # cov in [chunk, t] layout for new_coverage
            covT = rowpool.tile([CH, P], F32, tag="covT")
            nc.sync.dma_start(out=covT, in_=cov[r].rearrange("(c p) -> c p", p=P))

            e_row = rowpool.tile([P, CH], F32, tag="e_row")
            # whole-row h load (one DMA): hrow[p, c, d] = h[r, c*128+p, d]
            hrow = hrowpool.tile([P, CH, D], F32, tag="hrow")
            nc.sync.dma_start(
                out=hrow, in_=h[r].rearrange("(c p) d -> p c d", p=P)
            )
            for g in range(CH // QC):
                # quarter-row bf16 convert + one batched xbar transpose
                hbf = hbfpool.tile([P, QC, D], BF16, tag="hbf")
                nc.gpsimd.tensor_copy(out=hbf, in_=hrow[:, g * QC:(g + 1) * QC, :])
                ht = htpool.tile([P, QC * DB, P], BF16, tag="ht")
                nc.scalar.dma_start_transpose(
                    out=ht, in_=hbf.rearrange("p a b -> p (a b)")
                )
                tf4 = tfpool.tile([P, QC, D], BF16, tag="tf4")
                for q in range(QC):
                    c = g * QC + q
                    psf = psumA.tile([P, D], F32, tag="psf")
                    nc.tensor.matmul(
                        psf,
                        lhsT=cov2[:, c * P:(c + 1) * P],
                        rhs=rhs2[:, r * D:(r + 1) * D],
                        start=True, stop=False,
                    )
                    for j in range(DB):
                        nc.tensor.matmul(psf, lhsT=ht[:, q * DB + j, :],
                                         rhs=wht[j],
                                         start=False, stop=(j == DB - 1))
                    nc.scalar.activation(tf4[:, q, :], psf, AF.Tanh)
                sc4 = scpool.tile([P, QC, D], F32, tag="sc4")
                v_bc = bass.AP(
                    tensor=v_bf.tensor, offset=v_bf.offset,
                    ap=[list(v_bf.ap[0]), [0, QC], list(v_bf.ap[1])],
                )
                nc.vector.tensor_mul(sc4, tf4, v_bc)
                nc.vector.tensor_reduce(
                    out=e_row[:, g * QC:(g + 1) * QC], in_=sc4,
                    axis=mybir.AxisListType.X, op=ALU.add,
                )

            # softmax (no max-subtraction) + normalization scalars
            w_row = rowpool.tile([P, CH], F32, tag="w_row")
            s1 = rowpool.tile([P, 1], F32, tag="s1")
            nc.scalar.activation(w_row, e_row, AF.Exp, accum_out=s1)
            psum_z = psumS.tile([1, 1], F32, tag="z")
            nc.tensor.matmul(psum_z, lhsT=s1, rhs=ones_col, start=True, stop=True)
            rz = rowpool.tile([1, 1], F32, tag="rz")
            nc.vector.reciprocal(rz, psum_z)
            psum_rzc = psumS.tile([P, 1], F32, tag="rzc")
            nc.tensor.matmul(psum_rzc, lhsT=ones_row, rhs=rz, start=True, stop=True)
            rz_col = rowpool.tile([P, 1], F32, tag="rz_col")
            nc.scalar.copy(rz_col, psum_rzc)

            # context = (w @ h) / Z    (fp32)
            psum_ctx = psumC.tile([1, D], F32, tag="ctx")
            for c in range(CH):
                nc.tensor.matmul(psum_ctx, lhsT=w_row[:, c:c + 1],
                                 rhs=hrow[:, c, :],
                                 start=(c == 0), stop=(c == CH - 1))
            ctx_s = rowpool.tile([1, D], F32, tag="ctx_s")
            nc.scalar.activation(ctx_s, psum_ctx, AF.Copy, scale=rz)
            nc.sync.dma_start(out=octx[r:r + 1, :], in_=ctx_s)

            # a_t (transpose w to [chunk, t] layout, scale by 1/Z)
            psum_wT = psumS.tile([CH, P], F32, tag="wT")
            nc.tensor.transpose(psum_wT, w_row, ident)
            aT = rowpool.tile([CH, P], F32, tag="aT")
            nc.scalar.activation(aT, psum_wT, AF.Copy, scale=rz_col[0:CH, :])
            nc.sync.dma_start(out=oa[r].rearrange("(c p) -> c p", p=P), in_=aT)

            # new_coverage = coverage + a_t
            ncovT = rowpool.tile([CH, P], F32, tag="ncovT")
            nc.vector.tensor_add(ncovT, aT, covT)
            nc.sync.dma_start(out=oncov[r].rearrange("(c p) -> c p", p=P),
                              in_=ncovT)

    if reps == 1:
        emit_rows()
    else:
        with tc.For_i(0, reps, 1) as _i:
            emit_rows()


def build_nc(rows=ROWS, fix=True, reps=1):
    nc = bass.Bass("TRN2", target_bir_lowering=False, debug=False)
    h = nc.dram_tensor("h_i", [rows, T, D], F32, kind="ExternalInput").ap()
    s_t = nc.dram_tensor("s_t", [rows, 2 * D], F32, kind="ExternalInput").ap()
    cov = nc.dram_tensor("coverage", [rows, T], F32, kind="ExternalInput").ap()
    Wh = nc.dram_tensor("Wh_w", [D, D], F32, kind="ExternalInput").ap()
    Ws = nc.dram_tensor("Ws_w", [D, 2 * D], F32, kind="ExternalInput").ap()
    Wsb = nc.dram_tensor("Ws_b", [D], F32, kind="ExternalInput").ap()
    Wc = nc.dram_tensor("Wc_w", [D, 1], F32, kind="ExternalInput").ap()
    V = nc.dram_tensor("V_w", [1, D], F32, kind="ExternalInput").ap()
    octx = nc.dram_tensor("context", [rows, D], F32, kind="ExternalOutput").ap()
    oa = nc.dram_tensor("a_t", [rows, T], F32, kind="ExternalOutput").ap()
    oncov = nc.dram_tensor("new_coverage", [rows, T], F32,
                           kind="ExternalOutput").ap()

    with tile.TileContext(nc) as tc:
        with ExitStack() as ctx:
            build_body(nc, tc, ctx, rows, h, s_t, cov, Wh, Ws, Wsb, Wc, V,
                       octx, oa, oncov, reps=reps)
    if fix:
        fix_multiwait(nc)
    return nc


_NC_CACHE = {}


def kernel(h_i, s_t, coverage, Wh_w, Ws_w, Ws_b, Wc_w, V_w, **kw):
    h_i = np.ascontiguousarray(np.asarray(h_i, dtype=np.float32))
    s_t = np.ascontiguousarray(np.asarray(s_t, dtype=np.float32))
    coverage = np.ascontiguousarray(np.asarray(coverage, dtype=np.float32))
    shared = {
        "Wh_w": np.ascontiguousarray(np.asarray(Wh_w, dtype=np.float32)),
        "Ws_w": np.ascontiguousarray(np.asarray(Ws_w, dtype=np.float32)),
        "Ws_b": np.ascontiguousarray(np.asarray(Ws_b, dtype=np.float32)),
        "Wc_w": np.ascontiguousarray(np.asarray(Wc_w, dtype=np.float32)),
        "V_w": np.ascontiguousarray(np.asarray(V_w, dtype=np.float32)),
    }
    if "nc" not in _NC_CACHE:
        _NC_CACHE["nc"] = build_nc()
    nc = _NC_CACHE["nc"]
    in_maps = []
    for c in range(NCORES):
        sl = slice(c * ROWS, (c + 1) * ROWS)
        in_maps.append({
            "h_i": h_i[sl], "s_t": s_t[sl], "coverage": coverage[sl],
            **shared,
        })
    res = run_bass_kernel_spmd(nc, in_maps, core_ids=list(range(NCORES)))
    context = np.concatenate([res.results[c]["context"] for c in range(NCORES)], 0)
    a_t = np.concatenate([res.results[c]["a_t"] for c in range(NCORES)], 0)
    ncov = np.concatenate([res.results[c]["new_coverage"] for c in range(NCORES)], 0)
    return (context, a_t, ncov)


# revision 13
# speedup vs baseline: 1.0286x; 1.0286x over previous
"""Bahdanau attention w/ coverage — Trainium2 Bass kernel, 8 NeuronCores.

Data-parallel over batch: each core handles 8 rows of the batch.
Per row (T=2048, D=512), processed in 16 chunks of 128 timesteps:
  f[t,e]   = sum_d h[t,d]*Wh[e,d] + dec_b[e] + cov[t]*Wc[e]   (PSUM accum:
             one K=2 matmul for the rank-2 dec/cov part + 4 K=128 bf16
             matmuls with hT tiles obtained via DMA-xbar transpose)
  e_t[t]   = sum_e tanh(f[t,e]) * V[e]      (ACT tanh + DVE fused mul-reduce)
  a        = exp(e_t) / Z                    (no max-subtraction: logits are
             tiny by construction; Z via ones-matmul partition reduce)
  context  = (w @ h) / Z                     (fp32 PSUM accum over resident h)
h is read from HBM exactly once per core.
"""

import numpy as np
from contextlib import ExitStack

import concourse.bass as bass
import concourse.tile as tile
from concourse import mybir
from concourse.masks import make_identity
from concourse.bass_utils import run_bass_kernel_spmd

B, T, D = 64, 2048, 512
NCORES = 8
ROWS = B // NCORES   # 8 batch rows per core
P = 128
CH = T // P          # 16 chunks per row
DB = D // P          # 4 d-blocks
KB = (2 * D) // P    # 8 k-blocks of s_t
F32 = mybir.dt.float32
BF16 = mybir.dt.bfloat16
AF = mybir.ActivationFunctionType
ALU = mybir.AluOpType


def _bcast(ap, n):
    """Partition-broadcast a 1-D (or [1, N]) DRAM AP to n partitions."""
    while len(ap.shape) > 1 and ap.shape[0] == 1:
        ap = ap[0]
    return bass.AP(tensor=ap.tensor, offset=ap.offset, ap=[[0, n]] + list(ap.ap))


def fix_multiwait(nc):
    """This walrus build allows a single sem-wait per instruction; Tile's
    tail drain packs the whole residual vector clock onto one Drain.
    Hoist excess waits onto injected same-engine NoOps."""
    for f in nc.m.functions:
        for bb in f.blocks:
            insts = bb.instructions
            new = []
            changed = False
            for inst in insts:
                si = inst.sync_info
                if si is not None and si.on_wait and len(si.on_wait) > 1:
                    waits = list(si.on_wait)
                    for k, w in enumerate(waits[:-1]):
                        nop = mybir.InstNoOp(
                            name=f"{inst.name}-hoistw{k}", ins=[], outs=[]
                        )
                        nop.engine = inst.engine
                        nop.sync_info = mybir.SyncInfo(on_wait=[w], on_update=[])
                        new.append(nop)
                    si.on_wait = waits[-1:]
                    changed = True
                new.append(inst)
            if changed:
                bb.instructions = new


def build_body(nc, tc, ctx, rows, h, s_t, cov, Wh, Ws, Wsb, Wc, V,
               octx, oa, oncov, reps=1):
    consts = ctx.enter_context(tc.tile_pool(name="consts", bufs=1))
    stage = ctx.enter_context(tc.tile_pool(name="stage", bufs=2))
    psumA = ctx.enter_context(tc.tile_pool(name="psumA", bufs=4, space="PSUM"))
    psumC = ctx.enter_context(tc.tile_pool(name="psumC", bufs=1, space="PSUM"))
    psumS = ctx.enter_context(tc.tile_pool(name="psumS", bufs=1, space="PSUM"))

    # ---------------- one-time setup ----------------
    ident = consts.tile([P, P], F32, tag="ident")
    make_identity(nc, ident)
    ones_col = consts.tile([P, 1], F32, tag="ones_col")
    nc.vector.memset(ones_col, 1.0)
    ones_row = consts.tile([1, P], F32, tag="ones_row")
    nc.vector.memset(ones_row, 1.0)

    # V broadcast to all partitions, bf16
    v_f32 = consts.tile([P, D], F32, tag="v_f32")
    nc.sync.dma_start(out=v_f32, in_=_bcast(V, P))
    v_bf = consts.tile([P, D], BF16, tag="v_bf")
    nc.gpsimd.tensor_copy(out=v_bf, in_=v_f32)

    # WhT (bf16): WhT[j][d, e] with d in block j ; Wh[e, d] in HBM
    wht = [consts.tile([P, D], BF16, tag=f"wht{j}", name=f"wht{j}") for j in range(DB)]
    for i in range(DB):
        wh_stage = stage.tile([P, D], F32, tag="wh_stage")
        nc.sync.dma_start(out=wh_stage, in_=Wh[i * P:(i + 1) * P, :])
        wh_bf = stage.tile([P, D], BF16, tag="wh_bf")
        nc.gpsimd.tensor_copy(out=wh_bf, in_=wh_stage)
        for j in range(DB):
            nc.scalar.dma_start_transpose(
                out=wht[j][:, i * P:(i + 1) * P],
                in_=wh_bf[:, j * P:(j + 1) * P],
            )

    # WsT (bf16): WsT[k][kk, e]; Ws[e, k] in HBM  [512, 1024]
    wst = [consts.tile([P, D], BF16, tag=f"wst{k}", name=f"wst{k}") for k in range(KB)]
    for i in range(DB):
        ws_stage = stage.tile([P, 2 * D], F32, tag="ws_stage")
        nc.sync.dma_start(out=ws_stage, in_=Ws[i * P:(i + 1) * P, :])
        ws_bf = stage.tile([P, 2 * D], BF16, tag="ws_bf")
        nc.gpsimd.tensor_copy(out=ws_bf, in_=ws_stage)
        for k in range(KB):
            nc.scalar.dma_start_transpose(
                out=wst[k][:, i * P:(i + 1) * P],
                in_=ws_bf[:, k * P:(k + 1) * P],
            )

    # s_t transposed: [128k, rows] per k-block (tiny strided DMA), bf16
    stt_bf = []
    for k in range(KB):
        stt_f = stage.tile([P, rows], F32, tag="stt_f")
        nc.sync.dma_start(
            out=stt_f, in_=s_t[:, k * P:(k + 1) * P].rearrange("a b -> b a")
        )
        t_bf = consts.tile([P, rows], BF16, tag=f"stt{k}", name=f"stt{k}")
        nc.gpsimd.tensor_copy(out=t_bf, in_=stt_f)
        stt_bf.append(t_bf)

    # dec[b, e] = s_t[b] @ Ws.T + Ws_b   -> [rows, 512] fp32
    psum_dec = psumS.tile([rows, D], F32, tag="dec")
    for k in range(KB):
        nc.tensor.matmul(psum_dec, lhsT=stt_bf[k], rhs=wst[k],
                         start=(k == 0), stop=(k == KB - 1))
    wsb_bc = consts.tile([rows, D], F32, tag="wsb_bc")
    nc.sync.dma_start(out=wsb_bc, in_=_bcast(Wsb, rows))
    dec_f = consts.tile([rows, D], F32, tag="dec_f")
    nc.vector.tensor_add(dec_f, psum_dec, wsb_bc)
    dec_bf = consts.tile([rows, D], BF16, tag="dec_bf")
    nc.gpsimd.tensor_copy(out=dec_bf, in_=dec_f)

    # Wc row (bf16)
    wc_f = consts.tile([1, D], F32, tag="wc_f")
    nc.sync.dma_start(out=wc_f, in_=Wc.rearrange("a b -> b a"))
    wc_bf = consts.tile([1, D], BF16, tag="wc_bf")
    nc.gpsimd.tensor_copy(out=wc_bf, in_=wc_f)

    # rhs2_all[2, rows*D]: per-row moving operand of the K=2 rank-2 matmul
    # partition 0 = dec_b, partition 1 = WcT
    rhs2 = consts.tile([2, rows * D], BF16, tag="rhs2")
    for b in range(rows):
        nc.sync.dma_start(out=rhs2[0:1, b * D:(b + 1) * D], in_=dec_bf[b:b + 1, :])
        nc.sync.dma_start(out=rhs2[1:2, b * D:(b + 1) * D], in_=wc_bf)

    # main-loop pools (created after setup emission; QC = chunks per group)
    QC = 4
    hrowpool = ctx.enter_context(tc.tile_pool(name="hrow", bufs=2))
    hbfpool = ctx.enter_context(tc.tile_pool(name="hbf", bufs=3))
    htpool = ctx.enter_context(tc.tile_pool(name="ht", bufs=3))
    tfpool = ctx.enter_context(tc.tile_pool(name="tf", bufs=2))
    scpool = ctx.enter_context(tc.tile_pool(name="sc", bufs=1))
    rowpool = ctx.enter_context(tc.tile_pool(name="rowpool", bufs=2))

    def emit_rows():
        for r in range(rows):
            # cov2: stationary [2, T] for the K=2 matmul (row0 ones, row1 cov)
            cov2 = rowpool.tile([2, T], BF16, tag="cov2")
            nc.vector.memset(cov2[0:1, :], 1.0)
            cov_f = rowpool.tile([1, T], F32, tag="cov_f", bufs=1)
            nc.sync.dma_start(out=cov_f, in_=cov[r:r + 1, :])
            cov_bf = rowpool.tile([1, T], BF16, tag="cov_bf", bufs=1)
            nc.gpsimd.tensor_copy(out=cov_bf, in_=cov_f)
            nc.sync.dma_start(out=cov2[1:2, :], in_=cov_bf)
            # cov in [chunk, t] layout for new_coverage
            covT = rowpool.tile([CH, P], F32, tag="covT")
            nc.sync.dma_start(out=covT, in_=cov[r].rearrange("(c p) -> c p", p=P))

            e_row = rowpool.tile([P, CH], F32, tag="e_row")
            # whole-row h load (one DMA): hrow[p, c, d] = h[r, c*128+p, d]
            hrow = hrowpool.tile([P, CH, D], F32, tag="hrow")
            nc.sync.dma_start(
                out=hrow, in_=h[r].rearrange("(c p) d -> p c d", p=P)
            )
            HC = 8  # chunks per transpose batch (half row)
            ht_halves = []
            hbf_halves = []
            for g2 in range(CH // HC):
                # half-row bf16 convert + one batched xbar transpose
                hbf = hbfpool.tile([P, HC, D], BF16, tag="hbf")
                nc.gpsimd.tensor_copy(out=hbf,
                                      in_=hrow[:, g2 * HC:(g2 + 1) * HC, :])
                ht_h = htpool.tile([P, HC * DB, P], BF16, tag="ht",
                                   name="ht_h")
                nc.scalar.dma_start_transpose(
                    out=ht_h, in_=hbf.rearrange("p a b -> p (a b)")
                )
                ht_halves.append(ht_h)
                hbf_halves.append(hbf)
            for g in range(CH // QC):
                tf4 = tfpool.tile([P, QC, D], BF16, tag="tf4")
                for q in range(QC):
                    c = g * QC + q
                    ht = ht_halves[c // HC]
                    psf = psumA.tile([P, D], F32, tag="psf")
                    nc.tensor.matmul(
                        psf,
                        lhsT=cov2[:, c * P:(c + 1) * P],
                        rhs=rhs2[:, r * D:(r + 1) * D],
                        start=True, stop=False,
                    )
                    for j in range(DB):
                        nc.tensor.matmul(psf,
                                         lhsT=ht[:, (c % HC) * DB + j, :],
                                         rhs=wht[j],
                                         start=False, stop=(j == DB - 1))
                    nc.scalar.activation(tf4[:, q, :], psf, AF.Tanh)
                sc4 = scpool.tile([P, QC, D], BF16, tag="sc4")
                v_bc = bass.AP(
                    tensor=v_bf.tensor, offset=v_bf.offset,
                    ap=[list(v_bf.ap[0]), [0, QC], list(v_bf.ap[1])],
                )
                nc.vector.tensor_mul(sc4, tf4, v_bc)
                nc.vector.tensor_reduce(
                    out=e_row[:, g * QC:(g + 1) * QC], in_=sc4,
                    axis=mybir.AxisListType.X, op=ALU.add,
                )

            # softmax (no max-subtraction) + normalization scalars
            w_row = rowpool.tile([P, CH], F32, tag="w_row")
            s1 = rowpool.tile([P, 1], F32, tag="s1")
            nc.scalar.activation(w_row, e_row, AF.Exp, accum_out=s1)
            psum_z = psumS.tile([1, 1], F32, tag="small", name="psum_z", bufs=2)
            nc.tensor.matmul(psum_z, lhsT=s1, rhs=ones_col, start=True, stop=True)
            rz = rowpool.tile([1, 1], F32, tag="rz")
            nc.vector.reciprocal(rz, psum_z)
            psum_rzc = psumS.tile([P, 1], F32, tag="small", name="psum_rzc", bufs=2)
            nc.tensor.matmul(psum_rzc, lhsT=ones_row, rhs=rz, start=True, stop=True)
            rz_col = rowpool.tile([P, 1], F32, tag="rz_col")
            nc.vector.tensor_copy(out=rz_col, in_=psum_rzc)

            # context = (w @ h) / Z    (fp32)
            w_bf = rowpool.tile([P, CH], BF16, tag="w_bf")
            nc.gpsimd.tensor_copy(out=w_bf, in_=w_row)
            psum_ctx = psumC.tile([1, D], F32, tag="ctx")
            for c in range(CH):
                nc.tensor.matmul(psum_ctx,
                                 lhsT=w_bf[:, c:c + 1],
                                 rhs=hbf_halves[c // HC][:, c % HC, :],
                                 start=(c == 0), stop=(c == CH - 1))
            ctx_s = rowpool.tile([1, D], F32, tag="ctx_s")
            nc.vector.tensor_scalar_mul(ctx_s, psum_ctx, rz)
            nc.sync.dma_start(out=octx[r:r + 1, :], in_=ctx_s)

            # a_t (transpose w to [chunk, t] layout, scale by 1/Z)
            psum_wT = psumS.tile([CH, P], F32, tag="small", name="psum_wT", bufs=2)
            nc.tensor.transpose(psum_wT, w_row, ident)
            aT = rowpool.tile([CH, P], F32, tag="aT")
            nc.vector.tensor_scalar_mul(aT, psum_wT, rz_col[0:CH, :])
            nc.sync.dma_start(out=oa[r].rearrange("(c p) -> c p", p=P), in_=aT)

            # new_coverage = coverage + a_t
            ncovT = rowpool.tile([CH, P], F32, tag="ncovT")
            nc.vector.tensor_add(ncovT, aT, covT)
            nc.sync.dma_start(out=oncov[r].rearrange("(c p) -> c p", p=P),
                              in_=ncovT)

    if reps == 1:
        emit_rows()
    else:
        with tc.For_i(0, reps, 1) as _i:
            emit_rows()


def build_nc(rows=ROWS, fix=True, reps=1):
    nc = bass.Bass("TRN2", target_bir_lowering=False, debug=False)
    h = nc.dram_tensor("h_i", [rows, T, D], F32, kind="ExternalInput").ap()
    s_t = nc.dram_tensor("s_t", [rows, 2 * D], F32, kind="ExternalInput").ap()
    cov = nc.dram_tensor("coverage", [rows, T], F32, kind="ExternalInput").ap()
    Wh = nc.dram_tensor("Wh_w", [D, D], F32, kind="ExternalInput").ap()
    Ws = nc.dram_tensor("Ws_w", [D, 2 * D], F32, kind="ExternalInput").ap()
    Wsb = nc.dram_tensor("Ws_b", [D], F32, kind="ExternalInput").ap()
    Wc = nc.dram_tensor("Wc_w", [D, 1], F32, kind="ExternalInput").ap()
    V = nc.dram_tensor("V_w", [1, D], F32, kind="ExternalInput").ap()
    octx = nc.dram_tensor("context", [rows, D], F32, kind="ExternalOutput").ap()
    oa = nc.dram_tensor("a_t", [rows, T], F32, kind="ExternalOutput").ap()
    oncov = nc.dram_tensor("new_coverage", [rows, T], F32,
                           kind="ExternalOutput").ap()

    with tile.TileContext(nc) as tc:
        with ExitStack() as ctx:
            build_body(nc, tc, ctx, rows, h, s_t, cov, Wh, Ws, Wsb, Wc, V,
                       octx, oa, oncov, reps=reps)
    if fix:
        fix_multiwait(nc)
    return nc


_NC_CACHE = {}


def kernel(h_i, s_t, coverage, Wh_w, Ws_w, Ws_b, Wc_w, V_w, **kw):
    h_i = np.ascontiguousarray(np.asarray(h_i, dtype=np.float32))
    s_t = np.ascontiguousarray(np.asarray(s_t, dtype=np.float32))
    coverage = np.ascontiguousarray(np.asarray(coverage, dtype=np.float32))
    shared = {
        "Wh_w": np.ascontiguousarray(np.asarray(Wh_w, dtype=np.float32)),
        "Ws_w": np.ascontiguousarray(np.asarray(Ws_w, dtype=np.float32)),
        "Ws_b": np.ascontiguousarray(np.asarray(Ws_b, dtype=np.float32)),
        "Wc_w": np.ascontiguousarray(np.asarray(Wc_w, dtype=np.float32)),
        "V_w": np.ascontiguousarray(np.asarray(V_w, dtype=np.float32)),
    }
    if "nc" not in _NC_CACHE:
        _NC_CACHE["nc"] = build_nc()
    nc = _NC_CACHE["nc"]
    in_maps = []
    for c in range(NCORES):
        sl = slice(c * ROWS, (c + 1) * ROWS)
        in_maps.append({
            "h_i": h_i[sl], "s_t": s_t[sl], "coverage": coverage[sl],
            **shared,
        })
    res = run_bass_kernel_spmd(nc, in_maps, core_ids=list(range(NCORES)))
    context = np.concatenate([res.results[c]["context"] for c in range(NCORES)], 0)
    a_t = np.concatenate([res.results[c]["a_t"] for c in range(NCORES)], 0)
    ncov = np.concatenate([res.results[c]["new_coverage"] for c in range(NCORES)], 0)
    return (context, a_t, ncov)


# revision 15
# speedup vs baseline: 1.3225x; 1.2857x over previous
"""Bahdanau attention w/ coverage — Trainium2 Bass kernel, 8 NeuronCores.

Data-parallel over batch: each core handles 8 rows of the batch.
Per row (T=2048, D=512), processed in 16 chunks of 128 timesteps:
  f[t,e]   = sum_d h[t,d]*Wh[e,d] + dec_b[e] + cov[t]*Wc[e]   (PSUM accum:
             one K=2 matmul for the rank-2 dec/cov part + 4 K=128 bf16
             matmuls with hT tiles obtained via DMA-xbar transpose)
  e_t[t]   = sum_e tanh(f[t,e]) * V[e]      (ACT tanh + DVE fused mul-reduce)
  a        = exp(e_t) / Z                    (no max-subtraction: logits are
             tiny by construction; Z via ones-matmul partition reduce)
  context  = (w @ h) / Z                     (bf16 PE accum over the resident
             bf16 copy of h, scaled by 1/Z on the PSUM->SBUF copy)
h is read from HBM exactly once per core (one 4MB DMA per row); the bf16
conversion (GPSIMD) and blockwise xbar transpose (one 3D-output
dma_start_transpose per row) are whole-row batched to minimize instruction
count and xbar mode switches.
"""

import numpy as np
from contextlib import ExitStack

import concourse.bass as bass
import concourse.tile as tile
from concourse import mybir
from concourse.masks import make_identity
from concourse.bass_utils import run_bass_kernel_spmd

B, T, D = 64, 2048, 512
NCORES = 8
ROWS = B // NCORES   # 8 batch rows per core
P = 128
CH = T // P          # 16 chunks per row
DB = D // P          # 4 d-blocks
KB = (2 * D) // P    # 8 k-blocks of s_t
F32 = mybir.dt.float32
BF16 = mybir.dt.bfloat16
AF = mybir.ActivationFunctionType
ALU = mybir.AluOpType


def _bcast(ap, n):
    """Partition-broadcast a 1-D (or [1, N]) DRAM AP to n partitions."""
    while len(ap.shape) > 1 and ap.shape[0] == 1:
        ap = ap[0]
    return bass.AP(tensor=ap.tensor, offset=ap.offset, ap=[[0, n]] + list(ap.ap))


def fix_multiwait(nc):
    """This walrus build allows a single sem-wait per instruction; Tile's
    tail drain packs the whole residual vector clock onto one Drain.
    Hoist excess waits onto injected same-engine NoOps."""
    for f in nc.m.functions:
        for bb in f.blocks:
            insts = bb.instructions
            new = []
            changed = False
            for inst in insts:
                si = inst.sync_info
                if si is not None and si.on_wait and len(si.on_wait) > 1:
                    waits = list(si.on_wait)
                    for k, w in enumerate(waits[:-1]):
                        nop = mybir.InstNoOp(
                            name=f"{inst.name}-hoistw{k}", ins=[], outs=[]
                        )
                        nop.engine = inst.engine
                        nop.sync_info = mybir.SyncInfo(on_wait=[w], on_update=[])
                        new.append(nop)
                    si.on_wait = waits[-1:]
                    changed = True
                new.append(inst)
            if changed:
                bb.instructions = new


def build_body(nc, tc, ctx, rows, h, s_t, cov, Wh, Ws, Wsb, Wc, V,
               octx, oa, oncov, reps=1):
    consts = ctx.enter_context(tc.tile_pool(name="consts", bufs=1))
    stage_ctx = ExitStack()
    stage = stage_ctx.enter_context(tc.tile_pool(name="stage", bufs=2))
    psumA = ctx.enter_context(tc.tile_pool(name="psumA", bufs=4, space="PSUM"))
    psumC = ctx.enter_context(tc.tile_pool(name="psumC", bufs=1, space="PSUM"))
    psumS = ctx.enter_context(tc.tile_pool(name="psumS", bufs=1, space="PSUM"))

    # ---------------- one-time setup ----------------
    ident = consts.tile([P, P], F32, tag="ident")
    make_identity(nc, ident)
    ones_col = consts.tile([P, 1], F32, tag="ones_col")
    nc.vector.memset(ones_col, 1.0)
    ones_row = consts.tile([1, P], F32, tag="ones_row")
    nc.vector.memset(ones_row, 1.0)

    # V broadcast to all partitions, bf16
    v_f32 = consts.tile([P, D], F32, tag="v_f32")
    nc.sync.dma_start(out=v_f32, in_=_bcast(V, P))
    v_bf = consts.tile([P, D], BF16, tag="v_bf")
    nc.gpsimd.tensor_copy(out=v_bf, in_=v_f32)

    # WhT (bf16): WhT[j][d, e] with d in block j ; Wh[e, d] in HBM
    wht = [consts.tile([P, D], BF16, tag=f"wht{j}", name=f"wht{j}") for j in range(DB)]
    for i in range(DB):
        wh_stage = stage.tile([P, D], F32, tag="wh_stage")
        nc.sync.dma_start(out=wh_stage, in_=Wh[i * P:(i + 1) * P, :])
        wh_bf = stage.tile([P, D], BF16, tag="wh_bf")
        nc.gpsimd.tensor_copy(out=wh_bf, in_=wh_stage)
        for j in range(DB):
            nc.scalar.dma_start_transpose(
                out=wht[j][:, i * P:(i + 1) * P],
                in_=wh_bf[:, j * P:(j + 1) * P],
            )

    # WsT (bf16): WsT[k][kk, e]; Ws[e, k] in HBM  [512, 1024]
    wst = [consts.tile([P, D], BF16, tag=f"wst{k}", name=f"wst{k}") for k in range(KB)]
    for i in range(DB):
        ws_stage = stage.tile([P, 2 * D], F32, tag="ws_stage")
        nc.sync.dma_start(out=ws_stage, in_=Ws[i * P:(i + 1) * P, :])
        ws_bf = stage.tile([P, 2 * D], BF16, tag="ws_bf")
        nc.gpsimd.tensor_copy(out=ws_bf, in_=ws_stage)
        for k in range(KB):
            nc.scalar.dma_start_transpose(
                out=wst[k][:, i * P:(i + 1) * P],
                in_=ws_bf[:, k * P:(k + 1) * P],
            )

    # s_t transposed: [128k, rows] per k-block (tiny strided DMA), bf16
    stt_bf = []
    for k in range(KB):
        stt_f = stage.tile([P, rows], F32, tag="stt_f")
        nc.sync.dma_start(
            out=stt_f, in_=s_t[:, k * P:(k + 1) * P].rearrange("a b -> b a")
        )
        t_bf = consts.tile([P, rows], BF16, tag=f"stt{k}", name=f"stt{k}")
        nc.gpsimd.tensor_copy(out=t_bf, in_=stt_f)
        stt_bf.append(t_bf)

    # dec[b, e] = s_t[b] @ Ws.T + Ws_b   -> [rows, 512] fp32
    psum_dec = psumS.tile([rows, D], F32, tag="dec")
    for k in range(KB):
        nc.tensor.matmul(psum_dec, lhsT=stt_bf[k], rhs=wst[k],
                         start=(k == 0), stop=(k == KB - 1))
    wsb_bc = consts.tile([rows, D], F32, tag="wsb_bc")
    nc.sync.dma_start(out=wsb_bc, in_=_bcast(Wsb, rows))
    dec_f = consts.tile([rows, D], F32, tag="dec_f")
    nc.vector.tensor_add(dec_f, psum_dec, wsb_bc)
    dec_bf = consts.tile([rows, D], BF16, tag="dec_bf")
    nc.gpsimd.tensor_copy(out=dec_bf, in_=dec_f)

    # Wc row (bf16)
    wc_f = consts.tile([1, D], F32, tag="wc_f")
    nc.sync.dma_start(out=wc_f, in_=Wc.rearrange("a b -> b a"))
    wc_bf = consts.tile([1, D], BF16, tag="wc_bf")
    nc.gpsimd.tensor_copy(out=wc_bf, in_=wc_f)

    # rhs2_all[2, rows*D]: per-row moving operand of the K=2 rank-2 matmul
    # partition 0 = dec_b, partition 1 = WcT
    rhs2 = consts.tile([2, rows * D], BF16, tag="rhs2")
    for b in range(rows):
        nc.sync.dma_start(out=rhs2[0:1, b * D:(b + 1) * D], in_=dec_bf[b:b + 1, :])
        nc.sync.dma_start(out=rhs2[1:2, b * D:(b + 1) * D], in_=wc_bf)

    stage_ctx.close()
    # main-loop pools (created after setup emission; QC = chunks per group)
    QC = 4
    hrowpool = ctx.enter_context(tc.tile_pool(name="hrow", bufs=2))
    hbfpool = ctx.enter_context(tc.tile_pool(name="hbf", bufs=2))
    htpool = ctx.enter_context(tc.tile_pool(name="ht", bufs=2))
    tfpool = ctx.enter_context(tc.tile_pool(name="tf", bufs=2))
    scpool = ctx.enter_context(tc.tile_pool(name="sc", bufs=1))
    rowpool = ctx.enter_context(tc.tile_pool(name="rowpool", bufs=2))

    def emit_rows():
        for r in range(rows):
            # cov2: stationary [2, T] for the K=2 matmul (row0 ones, row1 cov)
            cov2 = rowpool.tile([2, T], BF16, tag="cov2")
            nc.vector.memset(cov2[0:1, :], 1.0)
            cov_f = rowpool.tile([1, T], F32, tag="cov_f", bufs=1)
            nc.sync.dma_start(out=cov_f, in_=cov[r:r + 1, :])
            cov_bf = rowpool.tile([1, T], BF16, tag="cov_bf", bufs=1)
            nc.gpsimd.tensor_copy(out=cov_bf, in_=cov_f)
            nc.sync.dma_start(out=cov2[1:2, :], in_=cov_bf)
            # cov in [chunk, t] layout for new_coverage
            covT = rowpool.tile([CH, P], F32, tag="covT")
            nc.sync.dma_start(out=covT, in_=cov[r].rearrange("(c p) -> c p", p=P))

            e_row = rowpool.tile([P, CH], F32, tag="e_row")
            # whole-row h load (one DMA): hrow[p, c, d] = h[r, c*128+p, d]
            hrow = hrowpool.tile([P, CH, D], F32, tag="hrow")
            nc.sync.dma_start(
                out=hrow, in_=h[r].rearrange("(c p) d -> p c d", p=P)
            )
            HC = 16  # chunks per transpose batch (full row)
            ht_halves = []
            hbf_halves = []
            for g2 in range(CH // HC):
                # half-row bf16 convert + one batched xbar transpose
                hbf = hbfpool.tile([P, HC, D], BF16, tag="hbf")
                nc.gpsimd.tensor_copy(out=hbf,
                                      in_=hrow[:, g2 * HC:(g2 + 1) * HC, :])
                ht_h = htpool.tile([P, HC * DB, P], BF16, tag="ht",
                                   name="ht_h")
                nc.scalar.dma_start_transpose(
                    out=ht_h, in_=hbf.rearrange("p a b -> p (a b)")
                )
                ht_halves.append(ht_h)
                hbf_halves.append(hbf)
            for g in range(CH // QC):
                tf4 = tfpool.tile([P, QC, D], BF16, tag="tf4")
                for q in range(QC):
                    c = g * QC + q
                    ht = ht_halves[c // HC]
                    psf = psumA.tile([P, D], F32, tag="psf")
                    nc.tensor.matmul(
                        psf,
                        lhsT=cov2[:, c * P:(c + 1) * P],
                        rhs=rhs2[:, r * D:(r + 1) * D],
                        start=True, stop=False,
                    )
                    for j in range(DB):
                        nc.tensor.matmul(psf,
                                         lhsT=ht[:, (c % HC) * DB + j, :],
                                         rhs=wht[j],
                                         start=False, stop=(j == DB - 1))
                    nc.scalar.activation(tf4[:, q, :], psf, AF.Tanh)
                sc4 = scpool.tile([P, QC, D], BF16, tag="sc4")
                v_bc = bass.AP(
                    tensor=v_bf.tensor, offset=v_bf.offset,
                    ap=[list(v_bf.ap[0]), [0, QC], list(v_bf.ap[1])],
                )
                nc.vector.tensor_mul(sc4, tf4, v_bc)
                nc.vector.tensor_reduce(
                    out=e_row[:, g * QC:(g + 1) * QC], in_=sc4,
                    axis=mybir.AxisListType.X, op=ALU.add,
                )

            # softmax (no max-subtraction) + normalization scalars
            w_row = rowpool.tile([P, CH], F32, tag="w_row")
            s1 = rowpool.tile([P, 1], F32, tag="s1")
            nc.scalar.activation(w_row, e_row, AF.Exp, accum_out=s1)
            psum_z = psumS.tile([1, 1], F32, tag="small", name="psum_z", bufs=2)
            nc.tensor.matmul(psum_z, lhsT=s1, rhs=ones_col, start=True, stop=True)
            rz = rowpool.tile([1, 1], F32, tag="rz")
            nc.vector.reciprocal(rz, psum_z)
            psum_rzc = psumS.tile([P, 1], F32, tag="small", name="psum_rzc", bufs=2)
            nc.tensor.matmul(psum_rzc, lhsT=ones_row, rhs=rz, start=True, stop=True)
            rz_col = rowpool.tile([P, 1], F32, tag="rz_col")
            nc.vector.tensor_copy(out=rz_col, in_=psum_rzc)

            # context = (w @ h) / Z    (fp32)
            w_bf = rowpool.tile([P, CH], BF16, tag="w_bf")
            nc.gpsimd.tensor_copy(out=w_bf, in_=w_row)
            psum_ctx = psumC.tile([1, D], F32, tag="ctx")
            for c in range(CH):
                nc.tensor.matmul(psum_ctx,
                                 lhsT=w_bf[:, c:c + 1],
                                 rhs=hbf_halves[c // HC][:, c % HC, :],
                                 start=(c == 0), stop=(c == CH - 1))
            ctx_s = rowpool.tile([1, D], F32, tag="ctx_s")
            nc.vector.tensor_scalar_mul(ctx_s, psum_ctx, rz)
            nc.sync.dma_start(out=octx[r:r + 1, :], in_=ctx_s)

            # a_t (transpose w to [chunk, t] layout, scale by 1/Z)
            psum_wT = psumS.tile([CH, P], F32, tag="small", name="psum_wT", bufs=2)
            nc.tensor.transpose(psum_wT, w_row, ident)
            aT = rowpool.tile([CH, P], F32, tag="aT")
            nc.vector.tensor_scalar_mul(aT, psum_wT, rz_col[0:CH, :])
            nc.sync.dma_start(out=oa[r].rearrange("(c p) -> c p", p=P), in_=aT)

            # new_coverage = coverage + a_t
            ncovT = rowpool.tile([CH, P], F32, tag="ncovT")
            nc.vector.tensor_add(ncovT, aT, covT)
            nc.sync.dma_start(out=oncov[r].rearrange("(c p) -> c p", p=P),
                              in_=ncovT)

    if reps == 1:
        emit_rows()
    else:
        with tc.For_i(0, reps, 1) as _i:
            emit_rows()


def build_nc(rows=ROWS, fix=True, reps=1):
    nc = bass.Bass("TRN2", target_bir_lowering=False, debug=False)
    h = nc.dram_tensor("h_i", [rows, T, D], F32, kind="ExternalInput").ap()
    s_t = nc.dram_tensor("s_t", [rows, 2 * D], F32, kind="ExternalInput").ap()
    cov = nc.dram_tensor("coverage", [rows, T], F32, kind="ExternalInput").ap()
    Wh = nc.dram_tensor("Wh_w", [D, D], F32, kind="ExternalInput").ap()
    Ws = nc.dram_tensor("Ws_w", [D, 2 * D], F32, kind="ExternalInput").ap()
    Wsb = nc.dram_tensor("Ws_b", [D], F32, kind="ExternalInput").ap()
    Wc = nc.dram_tensor("Wc_w", [D, 1], F32, kind="ExternalInput").ap()
    V = nc.dram_tensor("V_w", [1, D], F32, kind="ExternalInput").ap()
    octx = nc.dram_tensor("context", [rows, D], F32, kind="ExternalOutput").ap()
    oa = nc.dram_tensor("a_t", [rows, T], F32, kind="ExternalOutput").ap()
    oncov = nc.dram_tensor("new_coverage", [rows, T], F32,
                           kind="ExternalOutput").ap()

    with tile.TileContext(nc) as tc:
        with ExitStack() as ctx:
            build_body(nc, tc, ctx, rows, h, s_t, cov, Wh, Ws, Wsb, Wc, V,
                       octx, oa, oncov, reps=reps)
    if fix:
        fix_multiwait(nc)
    return nc


_NC_CACHE = {}


def kernel(h_i, s_t, coverage, Wh_w, Ws_w, Ws_b, Wc_w, V_w, **kw):
    h_i = np.ascontiguousarray(np.asarray(h_i, dtype=np.float32))
    s_t = np.ascontiguousarray(np.asarray(s_t, dtype=np.float32))
    coverage = np.ascontiguousarray(np.asarray(coverage, dtype=np.float32))
    shared = {
        "Wh_w": np.ascontiguousarray(np.asarray(Wh_w, dtype=np.float32)),
        "Ws_w": np.ascontiguousarray(np.asarray(Ws_w, dtype=np.float32)),
        "Ws_b": np.ascontiguousarray(np.asarray(Ws_b, dtype=np.float32)),
        "Wc_w": np.ascontiguousarray(np.asarray(Wc_w, dtype=np.float32)),
        "V_w": np.ascontiguousarray(np.asarray(V_w, dtype=np.float32)),
    }
    if "nc" not in _NC_CACHE:
        _NC_CACHE["nc"] = build_nc()
    nc = _NC_CACHE["nc"]
    in_maps = []
    for c in range(NCORES):
        sl = slice(c * ROWS, (c + 1) * ROWS)
        in_maps.append({
            "h_i": h_i[sl], "s_t": s_t[sl], "coverage": coverage[sl],
            **shared,
        })
    res = run_bass_kernel_spmd(nc, in_maps, core_ids=list(range(NCORES)))
    context = np.concatenate([res.results[c]["context"] for c in range(NCORES)], 0)
    a_t = np.concatenate([res.results[c]["a_t"] for c in range(NCORES)], 0)
    ncov = np.concatenate([res.results[c]["new_coverage"] for c in range(NCORES)], 0)
    return (context, a_t, ncov)
